# revision 19
# baseline (speedup 1.0000x reference)
"""Trainium2 Bass kernel for nn_FFTConv: y = tanh(Re(ifft(fft(u)*Ks)) + D*u).

v3 design:
  * Complex packing: conv with a REAL kernel commutes with Re/Im, so pack
    z[j] = u[2j] + i*u[2j+1] (j in [0,8)) per h -> halves all work. The
    real kernel's spectrum Ks (with D*u and the 1/L ifft scale folded in)
    is computed on the HOST from the 128 poles {A, conj(A)} and shipped as
    a bf16 parameter in its final on-chip layout -- no device prologue.
  * All matmuls bf16 (1 cyc/row), elementwise in bf16 SBUF (DVE 2x mode),
    full 128-partition layouts via block-diagonal stationaries for the
    64-point DFT stages. GPSIMD is avoided entirely (HW tensor ops there
    are ~20x slower than any model).
  * Fwd-twiddle add/sub folded into the corner-turn transposes via +/-I
    accumulation on the PE.
  * H-sharded over 8 cores (32 ch/core); per h: 8 packed complex rows,
    free dim 512 everywhere; NLANES software-pipelined lanes with skew.

Per-h dataflow (L = 8192 = 64*128, n = n1 + 128*n2, m = k2 + 64*k1):
  u:      [p=(g,n2), (c bb), n1]  one contiguous DMA (host pre-layout)
  stage1  (PE)  BD(F64) over n2        -> S1r,S1i [p=(g,k2), bb, n1]
  y1 copy (Act) PSUM->SBUF bf16 as [p, bb, (c n1)]
  fwd tw  (DVE) 2 paired mults ma=y1*[t2r|t2i], mb=y1*[t2i|t2r]
  transp  (PE)  16x 128x128, re = ma0.T - ma1.T, im = mb0.T + mb1.T
  y2t     (DVE) PSUM->SBUF bf16
  stage2  (PE)  F128 over n1           -> S2r,S2i [p=k1, (bb g), k2]
  x copy  (Act) PSUM->SBUF bf16 as [p, (bb g), (c k2)]
  spectral(DVE) 2 paired mults vs Ks[(r,i)|(i,r)] + 2 addsubs
  inv1    (PE)  conj(F128) over k1     -> Z1r,Z1i [p=o2, (bb g), k2]
  z1 copy (Act)
  inv tw  (DVE) 2 paired mults + 2 addsubs
  transp  (PE)  8x
  z2t     (Act) PSUM->SBUF bf16
  final   (PE)  BD(conj(F64)) over k2  -> Fr,Fi [p=(g,n2), bb, n1]
  tanh    (Act) from PSUM -> yo f32; one contiguous DMA out
"""
import os
import sys
import numpy as np

for p in ("/opt/trn_rl_repo", "/root/.axon_site/_ro/trn_rl_repo"):
    if os.path.isdir(p) and p not in sys.path:
        sys.path.append(p)

import ml_dtypes

BF16 = ml_dtypes.bfloat16

B, H, L, P = 16, 256, 8192, 64
NCORES = 8
HSH = H // NCORES          # 32 channels per core
NLANES = int(os.environ.get("KERNEL_NLANES", "4"))
REPEAT = int(os.environ.get("KERNEL_REPEAT", "1"))
MIDBUFS = int(os.environ.get("KERNEL_MIDBUFS", "2"))
IOBUFS = int(os.environ.get("KERNEL_IOBUFS", "2"))
PFBUFS = int(os.environ.get("KERNEL_PFBUFS", "2"))
PTBUFS = int(os.environ.get("KERNEL_PTBUFS", "0"))
SKEW = int(os.environ.get("KERNEL_SKEW", "2"))
YQPOOL = os.environ.get("KERNEL_YQPOOL", "0") == "1"

_CACHE = {}


def _tables():
    a64 = np.arange(64)
    a128 = np.arange(128)
    c64 = np.cos(2 * np.pi * np.outer(a64, a64) / 64)
    s64 = np.sin(2 * np.pi * np.outer(a64, a64) / 64)
    c128 = np.cos(2 * np.pi * np.outer(a128, a128) / 128)
    s128 = np.sin(2 * np.pi * np.outer(a128, a128) / 128)
    z64 = np.zeros((64, 64))

    def bd(x):
        return np.block([[x, z64], [z64, x]])

    k2v = a128 % 64
    thT = 2 * np.pi * np.outer(k2v, a128) / L        # [(g,k2), n1]
    thI = 2 * np.pi * np.outer(a128, a64) / L        # [o2, k2]
    t2r, t2i = np.cos(thT), -np.sin(thT)
    tir, tii = np.cos(thI), np.sin(thI)
    t = {
        "bd64r": bd(c64), "bd64i": bd(-s64), "bd64in": bd(s64),
        "f128r": c128, "f128i": -s128, "f128in": s128, "f128rn": -c128,
        "i128": np.eye(128), "i128n": -np.eye(128),
        # paired twiddle tables: [re|im] and [im|re] side by side
        "t2a": np.concatenate([t2r, t2i], axis=1),   # [128, 256]
        "t2b": np.concatenate([t2i, t2r], axis=1),
        "tia": np.concatenate([tir, tii], axis=1),   # [128, 128]
        "tib": np.concatenate([tii, tir], axis=1),
    }
    return {k: v.astype(BF16) for k, v in t.items()}


TBL_SHAPES = {
    "bd64r": [128, 128], "bd64i": [128, 128], "bd64in": [128, 128],
    "f128r": [128, 128], "f128i": [128, 128], "f128in": [128, 128],
    "f128rn": [128, 128],
    "i128": [128, 128], "i128n": [128, 128],
    "t2a": [128, 256], "t2b": [128, 256],
    "tia": [128, 128], "tib": [128, 128],
}


def _build(nc_mod):
    bass, tile, mybir, bacc = nc_mod
    dt = mybir.dt
    f32 = dt.float32
    bf16 = dt.bfloat16

    nc = bacc.Bacc("TRN2", target_bir_lowering=False, debug=False)
    AF = mybir.ActivationFunctionType
    OP = mybir.AluOpType

    # ---------------- DRAM parameters ----------------
    # u2/y2 host-relayouted: [p=(g,n2), h, (c bb), n1] -> one contiguous
    # full-width DMA per h. ks2 is the host-computed spectrum in its final
    # on-chip layout [k1, h, (r,i,i,r), k2].
    u_d = nc.declare_dram_parameter("u2_sh", [128, HSH, 8, 128], bf16, isOutput=False)
    y_d = nc.declare_dram_parameter("y2_sh", [128, HSH, 8, 128], f32, isOutput=True)
    ks2_d = nc.declare_dram_parameter("ks2_sh", [128, HSH, 4, 64], bf16, isOutput=False)
    tbl_d = {n: nc.declare_dram_parameter(n, shp, bf16, isOutput=False)
             for n, shp in TBL_SHAPES.items()}

    with tile.TileContext(nc) as tc:
        with tc.tile_pool(name="const", bufs=1) as cpool:
            tb = {}
            for n in TBL_SHAPES:
                tb[n] = cpool.tile(TBL_SHAPES[n], bf16, tag=f"c_{n}", name=f"c_{n}")
                nc.sync.dma_start(tb[n][:], tbl_d[n][:])
            ks_all = cpool.tile([128, HSH, 4, 64], bf16, tag="ks_all")
            nc.sync.dma_start(
                ks_all[:].rearrange("p a b c -> p (a b c)"),
                ks2_d[:].rearrange("p a b c -> p (a b c)"))

            main_pools = [
                tc.tile_pool(name="io", bufs=IOBUFS),
                tc.tile_pool(name="mid", bufs=MIDBUFS),
                tc.tile_pool(name="pf", bufs=PFBUFS, space=bass.MemorySpace.PSUM),
            ]
            if PTBUFS > 0:
                main_pools.append(
                    tc.tile_pool(name="pt", bufs=PTBUFS, space=bass.MemorySpace.PSUM))
            iop = main_pools[0].__enter__()
            midp = main_pools[1].__enter__()
            pfp = main_pools[2].__enter__()
            ptp = main_pools[3].__enter__() if PTBUFS > 0 else None

            t2a_b = tb["t2a"][:].unsqueeze(1).broadcast_to([128, 4, 256])
            t2b_b = tb["t2b"][:].unsqueeze(1).broadcast_to([128, 4, 256])
            tia_b = tb["tia"][:].unsqueeze(1).broadcast_to([128, 8, 128])
            tib_b = tb["tib"][:].unsqueeze(1).broadcast_to([128, 8, 128])

            def stages(h, lane):
                sfx = str(lane)

                ks_a = ks_all[:, h, 0:2, :].rearrange("p a b -> p (a b)").unsqueeze(
                    1).broadcast_to([128, 8, 128])
                ks_b = ks_all[:, h, 2:4, :].rearrange("p a b -> p (a b)").unsqueeze(
                    1).broadcast_to([128, 8, 128])

                # --- load: one contiguous full-width DMA ---
                uc = iop.tile([128, 2, 512], bf16, tag="uc" + sfx, name="uc")
                nc.sync.dma_start(
                    uc[:].rearrange("p c (bb n1) -> p (c bb) n1", n1=128), u_d[:, h])
                yield

                # --- stage 1: BD(F64) over n2 ---
                S1i = pfp.tile([128, 512], f32, tag="pf" + sfx, name="S1i")
                S1r = pfp.tile([128, 512], f32, tag="pf" + sfx, name="S1r")
                ure, uim = uc[:, 0], uc[:, 1]
                nc.tensor.matmul(S1i[:], tb["bd64i"][:], ure, start=True, stop=False)
                nc.tensor.matmul(S1i[:], tb["bd64r"][:], uim, start=False, stop=True)
                nc.tensor.matmul(S1r[:], tb["bd64r"][:], ure, start=True, stop=False)
                nc.tensor.matmul(S1r[:], tb["bd64in"][:], uim, start=False, stop=True)
                # y1 layout [p, bb, (c n1)]
                y1 = midp.tile([128, 4, 256], bf16, tag="y1" + sfx, name="y1")
                nc.scalar.activation(
                    y1[:, :, 0:128], S1r[:].rearrange("p (a b) -> p a b", a=4), AF.Copy)
                nc.scalar.activation(
                    y1[:, :, 128:256], S1i[:].rearrange("p (a b) -> p a b", a=4), AF.Copy)
                yield

                # --- fwd twiddle: 2 paired mults + 2 addsubs (DVE) ---
                ma = midp.tile([128, 4, 256], bf16, tag="mA" + sfx, name="ma")
                mb = midp.tile([128, 4, 256], bf16, tag="mB" + sfx, name="mb")
                nc.vector.tensor_tensor(ma[:], y1[:], t2a_b, OP.mult)
                nc.vector.tensor_tensor(mb[:], y1[:], t2b_b, OP.mult)
                y2r = midp.tile([128, 512], bf16, tag="w1" + sfx, name="y2r")
                y2i = midp.tile([128, 512], bf16, tag="w2" + sfx, name="y2i")
                nc.vector.tensor_tensor(
                    y2r[:].rearrange("p (a b) -> p a b", a=4),
                    ma[:, :, 0:128], ma[:, :, 128:256], OP.subtract)
                nc.vector.tensor_tensor(
                    y2i[:].rearrange("p (a b) -> p a b", a=4),
                    mb[:, :, 0:128], mb[:, :, 128:256], OP.add)
                yield

                # --- fwd transposes ---
                TP = ptp.tile([128, 2, 512], bf16, tag="pt" + sfx, name="TP") \
                    if ptp is not None else \
                    pfp.tile([128, 2, 512], bf16, tag="pf" + sfx, name="TP")
                for cc, src in ((0, y2r), (1, y2i)):
                    tpv = TP[:, cc].rearrange("p (a b) -> p a b", a=4)
                    srcv = src[:].rearrange("p (a b) -> p a b", a=4)
                    for bb in range(4):
                        nc.tensor.transpose(tpv[:, bb, :], srcv[:, bb, :], tb["i128"][:])
                y2t = midp.tile([128, 2, 512], bf16, tag="y2t" + sfx, name="y2t")
                nc.vector.tensor_scalar_add(
                    y2t[:].rearrange("p a b -> p (a b)"),
                    TP[:].rearrange("p a b -> p (a b)"), 0.0)
                yield

                # --- stage 2: F128 over n1 ---
                S2i = pfp.tile([128, 512], f32, tag="pf" + sfx, name="S2i")
                S2r = pfp.tile([128, 512], f32, tag="pf" + sfx, name="S2r")
                y2tr, y2ti = y2t[:, 0], y2t[:, 1]
                nc.tensor.matmul(S2i[:], tb["f128i"][:], y2tr, start=True, stop=False)
                nc.tensor.matmul(S2i[:], tb["f128r"][:], y2ti, start=False, stop=True)
                nc.tensor.matmul(S2r[:], tb["f128r"][:], y2tr, start=True, stop=False)
                nc.tensor.matmul(S2r[:], tb["f128in"][:], y2ti, start=False, stop=True)
                # x layout [p, (bb g), (c k2)]
                x = midp.tile([128, 8, 128], bf16, tag="x" + sfx, name="x")
                nc.scalar.activation(
                    x[:, :, 0:64], S2r[:].rearrange("p (a b) -> p a b", a=8), AF.Copy)
                nc.scalar.activation(
                    x[:, :, 64:128], S2i[:].rearrange("p (a b) -> p a b", a=8), AF.Copy)
                yield

                # --- spectral multiply: 4 mults (DVE); addsubs fold into inv1 ---
                m1 = midp.tile([128, 512], bf16, tag="mA" + sfx, name="m1")
                m2 = midp.tile([128, 512], bf16, tag="mB" + sfx, name="m2")
                m3 = midp.tile([128, 512], bf16, tag="w1" + sfx, name="m3")
                m4 = midp.tile([128, 512], bf16, tag="w2" + sfx, name="m4")
                ksr_b = ks_all[:, h, 0, :].unsqueeze(1).broadcast_to([128, 8, 64])
                ksi_b = ks_all[:, h, 1, :].unsqueeze(1).broadcast_to([128, 8, 64])

                def v8(t):
                    return t[:].rearrange("p (a b) -> p a b", a=8)

                xr, xi = x[:, :, 0:64], x[:, :, 64:128]
                nc.vector.tensor_tensor(v8(m1), xr, ksr_b, OP.mult)
                nc.vector.tensor_tensor(v8(m2), xi, ksi_b, OP.mult)
                nc.vector.tensor_tensor(v8(m3), xr, ksi_b, OP.mult)
                nc.vector.tensor_tensor(v8(m4), xi, ksr_b, OP.mult)
                yield

                # --- inverse stage 1: conj(F128) @ (m1-m2, m3+m4) expanded ---
                Z1i = pfp.tile([128, 512], f32, tag="pf" + sfx, name="Z1i")
                Z1r = pfp.tile([128, 512], f32, tag="pf" + sfx, name="Z1r")
                nc.tensor.matmul(Z1i[:], tb["f128in"][:], m1[:], start=True, stop=False)
                nc.tensor.matmul(Z1i[:], tb["f128i"][:], m2[:], start=False, stop=False)
                nc.tensor.matmul(Z1i[:], tb["f128r"][:], m3[:], start=False, stop=False)
                nc.tensor.matmul(Z1i[:], tb["f128r"][:], m4[:], start=False, stop=True)
                nc.tensor.matmul(Z1r[:], tb["f128r"][:], m1[:], start=True, stop=False)
                nc.tensor.matmul(Z1r[:], tb["f128rn"][:], m2[:], start=False, stop=False)
                nc.tensor.matmul(Z1r[:], tb["f128i"][:], m3[:], start=False, stop=False)
                nc.tensor.matmul(Z1r[:], tb["f128i"][:], m4[:], start=False, stop=True)
                # z1 layout [p, (bb g), (c k2)]
                z1 = midp.tile([128, 8, 128], bf16, tag="z1" + sfx, name="z1")
                nc.scalar.activation(
                    z1[:, :, 0:64], Z1r[:].rearrange("p (a b) -> p a b", a=8), AF.Copy)
                nc.scalar.activation(
                    z1[:, :, 64:128], Z1i[:].rearrange("p (a b) -> p a b", a=8), AF.Copy)
                yield

                # --- inverse twiddle: 2 paired mults + 2 addsubs (DVE) ---
                ma = midp.tile([128, 8, 128], bf16, tag="mA" + sfx, name="ma")
                mb = midp.tile([128, 8, 128], bf16, tag="mB" + sfx, name="mb")
                nc.vector.tensor_tensor(ma[:], z1[:], tia_b, OP.mult)
                nc.vector.tensor_tensor(mb[:], z1[:], tib_b, OP.mult)
                z2r = midp.tile([128, 512], bf16, tag="w1" + sfx, name="z2r")
                z2i = midp.tile([128, 512], bf16, tag="w2" + sfx, name="z2i")
                nc.vector.tensor_tensor(
                    z2r[:].rearrange("p (a b) -> p a b", a=8),
                    ma[:, :, 0:64], ma[:, :, 64:128], OP.subtract)
                nc.vector.tensor_tensor(
                    z2i[:].rearrange("p (a b) -> p a b", a=8),
                    mb[:, :, 0:64], mb[:, :, 64:128], OP.add)
                yield

                # --- inverse transposes ---
                TQ = ptp.tile([128, 2, 512], bf16, tag="pt" + sfx, name="TQ") \
                    if ptp is not None else \
                    pfp.tile([128, 2, 512], bf16, tag="pf" + sfx, name="TQ")
                for cc, zsrc in ((0, z2r), (1, z2i)):
                    tqv = TQ[:, cc].rearrange("p (a b) -> p a b", a=4)
                    zsv = zsrc[:].rearrange("p (a b) -> p a b", a=4)
                    for bb in range(4):
                        nc.tensor.transpose(tqv[:, bb, :], zsv[:, bb, :], tb["i128"][:])
                z2t = midp.tile([128, 2, 512], bf16, tag="z2t" + sfx, name="z2t")
                nc.scalar.activation(
                    z2t[:].rearrange("p a b -> p (a b)"),
                    TQ[:].rearrange("p a b -> p (a b)"), AF.Copy)
                yield

                # --- final: BD(conj(F64)) over k2, tanh, store ---
                Fi = pfp.tile([128, 512], f32, tag="pf" + sfx, name="Fi")
                Fr = pfp.tile([128, 512], f32, tag="pf" + sfx, name="Fr")
                z2tr, z2ti = z2t[:, 0], z2t[:, 1]
                nc.tensor.matmul(Fi[:], tb["bd64in"][:], z2tr, start=True, stop=False)
                nc.tensor.matmul(Fi[:], tb["bd64r"][:], z2ti, start=False, stop=True)
                nc.tensor.matmul(Fr[:], tb["bd64r"][:], z2tr, start=True, stop=False)
                nc.tensor.matmul(Fr[:], tb["bd64i"][:], z2ti, start=False, stop=True)
                yo = iop.tile([128, 2, 512], f32, tag="yo" + sfx, name="yo")
                nc.scalar.activation(yo[:, 0], Fr[:], AF.Tanh)
                nc.scalar.activation(yo[:, 1], Fi[:], AF.Tanh)
                (nc.gpsimd if YQPOOL else nc.sync).dma_start(
                    y_d[:, h], yo[:].rearrange("p c (bb n1) -> p (c bb) n1", n1=128))
                yield

            def lane_stream(ln):
                for _rep in range(REPEAT):
                    for h in range(ln, HSH, NLANES):
                        yield from stages(h, ln)

            gens = [lane_stream(ln) for ln in range(NLANES)]
            done = [False] * NLANES
            # prime lanes with a stage skew so engine queues interleave
            # different pipeline stages instead of running in lockstep
            for ln in range(NLANES):
                for _ in range((NLANES - 1 - ln) * SKEW):
                    try:
                        next(gens[ln])
                    except StopIteration:
                        done[ln] = True
                        break
            while not all(done):
                for gi_, g in enumerate(gens):
                    if not done[gi_]:
                        try:
                            next(g)
                        except StopIteration:
                            done[gi_] = True

            for mp in reversed(main_pools):
                mp.__exit__(None, None, None)

    nc.compile()
    return nc


def _get_program():
    key = ("prog", NLANES, REPEAT, MIDBUFS, IOBUFS, PFBUFS, PTBUFS, SKEW, YQPOOL)
    if key not in _CACHE:
        import concourse.bass as bass
        import concourse.tile as tile
        from concourse import mybir, bacc
        _CACHE[key] = _build((bass, tile, mybir, bacc))
    return _CACHE[key]


def _u_relayout(u_bf_core):
    """[16, HSH, L] -> [p=(g,n2), h, (c bb), n1]  (b = g*8 + bb*2 + c)."""
    us = u_bf_core.reshape(2, 4, 2, HSH, 64, 128)   # g bb c h n2 n1
    return np.ascontiguousarray(us.transpose(0, 4, 3, 2, 1, 5).reshape(128, HSH, 8, 128))


def _y_relayout(y2_core):
    """[p=(g,n2), h, (c bb), n1] -> [16, HSH, L]."""
    ys = y2_core.reshape(2, 64, HSH, 2, 4, 128)      # g n2 h c bb n1
    return ys.transpose(0, 4, 3, 2, 1, 5).reshape(16, HSH, L)


def make_in_maps(u, A_re, A_im, BC_re, BC_im, D):
    u = np.asarray(u, dtype=np.float32)
    A_re = np.asarray(A_re, dtype=np.float32)
    A_im = np.asarray(A_im, dtype=np.float32)
    BC_re = np.asarray(BC_re, dtype=np.float32)
    BC_im = np.asarray(BC_im, dtype=np.float32)
    D = np.asarray(D, dtype=np.float32)
    tabs = _tables()
    u_bf = np.ascontiguousarray(u).astype(BF16)
    # host-side kernel spectrum: Ks[h,m] = (sum_p c'_p/(1-A'_p W^m) + D[h])/L
    m = np.arange(L)
    W = np.exp(-2j * np.pi * m / L).astype(np.complex64)
    A2 = np.concatenate([A_re + 1j * A_im, A_re - 1j * A_im]).astype(np.complex64)
    G = (1.0 / (1.0 - A2[:, None] * W[None, :])).astype(np.complex64)   # (128, L)
    BC = (BC_re + 1j * BC_im).astype(np.complex64)
    in_maps = []
    for c in range(NCORES):
        hs = slice(c * HSH, (c + 1) * HSH)
        C2 = np.concatenate([BC[hs] / 2, np.conj(BC[hs]) / 2], axis=1)  # (HSH, 128)
        Ks = (C2 @ G + D[hs].astype(np.complex64)[:, None]) / np.float32(L)  # (HSH, L)
        kr = Ks.real.astype(BF16).reshape(HSH, 128, 64)   # [h, k1, k2]
        ki = Ks.imag.astype(BF16).reshape(HSH, 128, 64)
        ks2 = np.stack([kr, ki, ki, kr], axis=2)          # [h, k1, c, k2]
        ks2 = np.ascontiguousarray(ks2.transpose(1, 0, 2, 3))  # [k1, h, c, k2]
        m_ = {
            "u2_sh": _u_relayout(u_bf[:, hs, :]),
            "ks2_sh": ks2,
        }
        m_.update(tabs)
        in_maps.append(m_)
    return in_maps


def kernel(u, A_re, A_im, BC_re, BC_im, D):
    from concourse.bass_utils import run_bass_kernel_spmd

    nc = _get_program()
    in_maps = make_in_maps(u, A_re, A_im, BC_re, BC_im, D)

    res = None
    last_err = None
    for attempt in range(3):
        try:
            res = run_bass_kernel_spmd(nc, in_maps, list(range(NCORES)))
            break
        except Exception as e:  # transient NRT_EXEC_UNIT_UNRECOVERABLE flakes
            last_err = e
            import time as _time
            _time.sleep(2.0)
    if res is None:
        raise last_err
    out = np.concatenate(
        [_y_relayout(res.results[c]["y2_sh"]) for c in range(NCORES)], axis=1)
    return np.ascontiguousarray(out, dtype=np.float32)


if __name__ == "__main__":
    rng = np.random.default_rng(0)
    u = rng.standard_normal((B, H, L), dtype=np.float32)
    A_re = rng.uniform(0.5, 0.99, P).astype(np.float32)
    A_im = rng.uniform(-0.5, 0.5, P).astype(np.float32)
    BC_re = rng.standard_normal((H, P), dtype=np.float32)
    BC_im = rng.standard_normal((H, P), dtype=np.float32)
    D = rng.uniform(0, 1, H).astype(np.float32)
    y = kernel(u=u, A_re=A_re, A_im=A_im, BC_re=BC_re, BC_im=BC_im, D=D)
    print("out", y.shape, y.dtype)
